# revision 14
# baseline (speedup 1.0000x reference)
"""Trainium2 Bass kernel for DeBERTa-style disentangled attention.

Problem: B=16, S=512, D=768, H=12, HD=64, L=512 (att_span), scale=sqrt(3*64).

  Q = q@Wq+bq, K = k@Wk+bk, V = v@Wv+bv   (per-head split)
  scores = (QK^T + c2p + p2c) / scale ; softmax ; ctx = P@V
  c2p[i,j] = Q[i] . pos_k[i-j+512]   (pos_k = rel@Wk+bk, per head)
  p2c[i,j] = K[j] . pos_q[i-j+512]   (pos_q = rel@Wq+bq)
  (clip never binds: i-j+512 in [1,1023])

Sharding: data-parallel over batch, 8 cores x (B_local=2).

Device strategy (per core, everything transposed "scores^T[j,i]"):
  - Projections produce QT/KT [dout, tok] (bf16), V [tok, dout] (bf16,
    augmented with a ones-column per head for softmax denominators),
    PKR = pos_k_reversed^T and PQ = pos_q^T [dout, p] (bf16).
    1/scale folded into Wq/bq on host (covers c2c, c2p via Q; p2c via pos_q).
  - Per (b,h): band matmuls produce c2p_att_rev / p2c_att [128, 640] tiles,
    evacuated bf16 and round-tripped through DRAM; strided re-read
    (row-stride 639, offset 127) yields the diagonal-gathered
    c2p [i,j] / p2cT [j,i] tiles (the DeBERTa "skew" trick).
  - scores^T accumulated in PSUM: c2cT matmul + p2cT via identity-add +
    c2p via PE add-transpose (lhsT=c2p chunk, rhs=identity).
  - exp on ACT (no max subtraction needed: |scores| <~ 3), PV matmul with
    ones-augmented V gives ctx^T and row sums; PE transpose + DVE
    reciprocal/scale finishes ctx = P@V / sums in fp32.
"""

import os
import sys
import numpy as np

for p in ("/opt/trn_rl_repo",):
    if p not in sys.path:
        sys.path.insert(0, p)

import ml_dtypes

import concourse.bass as bass
import concourse.bacc as bacc
import concourse.tile as tile
import concourse.mybir as mybir
from concourse import bass_utils

f32 = mybir.dt.float32
bf16 = mybir.dt.bfloat16
fp8 = mybir.dt.float8e4
FT = mybir.ActivationFunctionType

B, S, D, H = 16, 512, 768, 12
HD = 64
L = 512
P2 = 2 * L  # 1024
NB = 2  # batches per core
NTOK = NB * S  # 1024
NCORES = 8
SCALE = float(np.sqrt(HD * 3.0))
KC = D // 128  # 6 contraction chunks
BW = 640  # band width (pads the 639 used diagonals)
BP = 639  # band row pitch for the skew read

_nbf = ml_dtypes.bfloat16

# ablation / tuning knobs (TimelineSim experiments)
_ABL = set(os.environ.get("KABL", "").split(",")) - {""}
_BUFS = {}
for _kv in os.environ.get("KBUFS", "").split(","):
    if _kv:
        _k, _v = _kv.split("=")
        _BUFS[_k] = int(_v)


def _bufs(name, default):
    return _BUFS.get(name, default)


def build_kernel(abl=None, bufs=None, nrep=1):
    global _ABL, _BUFS
    if abl is not None:
        _ABL = set(abl)
    if bufs is not None:
        _BUFS = dict(bufs)
    nc = bacc.Bacc(
        "TRN2",
        target_bir_lowering=False,
        debug=False,
        enable_asserts=False,
        num_devices=NCORES,
    )

    # ---- I/O ----
    qT_d = nc.dram_tensor("qT", [D, NTOK], bf16, kind="ExternalInput")
    kT_d = nc.dram_tensor("kT", [D, NTOK], bf16, kind="ExternalInput")
    vT_d = nc.dram_tensor("vT", [D, NTOK], bf16, kind="ExternalInput")
    wq_d = nc.dram_tensor("Wq", [D, D], bf16, kind="ExternalInput")  # pre-scaled
    wk_d = nc.dram_tensor("Wk", [D, D], bf16, kind="ExternalInput")
    wv_d = nc.dram_tensor("Wv", [D, D], bf16, kind="ExternalInput")
    rT_d = nc.dram_tensor("rT", [D, P2], bf16, kind="ExternalInput")
    rTr_d = nc.dram_tensor("rTrev", [D, P2], bf16, kind="ExternalInput")
    bias_d = nc.dram_tensor("biases", [128, 2 * KC], f32, kind="ExternalInput")
    # ctx^T per (batch, head): [65, 512] (64 dims + exp-sum row);
    # host divides by sums and transposes
    out_d = nc.dram_tensor("out", [NB * H * 65, 512], f32,
                           kind="ExternalOutput")

    idn_np = np.eye(128, dtype=_nbf)
    if "fp8rt" in _ABL:
        idn_np = np.eye(128, dtype=ml_dtypes.float8_e4m3)
    idn_d = nc.inline_tensor(idn_np, name="idn_bf")

    with tile.TileContext(nc) as tc:
        for _rep in range(nrep):
            _body(nc, tc, qT_d, kT_d, vT_d, wq_d, wk_d, wv_d, rT_d, rTr_d,
                  bias_d, idn_d, out_d)
    nc.compile()
    return nc


def _body(nc, tc, qT_d, kT_d, vT_d, wq_d, wk_d, wv_d, rT_d, rTr_d,
          bias_d, idn_d, out_d):
    from contextlib import ExitStack

    with ExitStack() as big:
        const = big.enter_context(tc.tile_pool(name="const", bufs=1))
        acts = big.enter_context(tc.tile_pool(name="acts", bufs=1))

        bdt = fp8 if "fp8rt" in _ABL else bf16
        idn = const.tile([128, 128], bdt)
        nc.sync.dma_start(idn[:], idn_d.ap())
        biases = const.tile([128, 2 * KC], f32)
        nc.sync.dma_start(biases[:], bias_d.ap())

        # persistent activations
        QT = [acts.tile([128, NTOK], bf16, name=f"QT{t}") for t in range(KC)]
        KT = [acts.tile([128, NTOK], bf16, name=f"KT{t}") for t in range(KC)]
        PKR = [acts.tile([128, P2 + 1], bf16, name=f"PKR{t}") for t in range(KC)]
        PQ = [acts.tile([128, P2 + 1], bf16, name=f"PQ{t}") for t in range(KC)]
        VA = [acts.tile([128, 65 * H], bf16, name=f"VA{c}") for c in range(8)]

        # ---------------- Stage P: projections ----------------
        with ExitStack() as st:
            inp = st.enter_context(tc.tile_pool(name="inp", bufs=1))
            psp = st.enter_context(
                tc.tile_pool(name="psp", bufs=4, space="PSUM"))

            # merged input tiles: one DMA per tensor; chunk kc lives at
            # free-dim offset kc*ncols (src dims ordered (p, kc, col))
            def _load_merged(name, src_d, ncols):
                t_ = inp.tile([128, KC * ncols], bf16, name=name)
                src = bass.AP(src_d, 0,
                              [[ncols, 128], [128 * ncols, KC], [1, ncols]])
                nc.sync.dma_start(t_[:], src)
                return [t_[:, ncols * kc: ncols * (kc + 1)] for kc in range(KC)]

            # load order: weights first so the Q/K projections can start
            # while the rest of the inputs stream in; V path last (consumed
            # latest)
            wq = _load_merged("wqm", wq_d, D)
            wk = _load_merged("wkm", wk_d, D)
            qT = _load_merged("qTm", qT_d, NTOK)
            kTt = _load_merged("kTm", kT_d, NTOK)
            rTr = _load_merged("rTrm", rTr_d, P2)
            rT = _load_merged("rTm", rT_d, P2)
            wv = _load_merged("wvm", wv_d, D)
            vT = _load_merged("vTm", vT_d, NTOK)

            # QT / KT / PKR / PQ : out[dout_tile, tok] = W^T @ xT (+ bias)
            if "noproj" in _ABL:
                for t in range(KC):
                    nc.vector.memset(QT[t][:], 0.0)
                    nc.vector.memset(KT[t][:], 0.0)
                    nc.vector.memset(PKR[t][:], 0.0)
                    nc.vector.memset(PQ[t][:], 0.0)
                for c in range(8):
                    nc.vector.memset(VA[c][:], 0.0)
            for t in range(KC if "noproj" not in _ABL else 0):
                wsl = slice(128 * t, 128 * (t + 1))
                for th in range(2):  # token/pos halves of 512
                    tsl = slice(512 * th, 512 * (th + 1))
                    for (wmat, xin, bcol, dst) in (
                        (wq, qT, 0, QT), (wk, kTt, 1, KT),
                        (wk, rTr, 1, PKR), (wq, rT, 0, PQ),
                    ):
                        ps = psp.tile([128, 512], f32, name="ps_proj",
                                      tag="ps_proj", bufs=4)
                        for kc in range(KC):
                            nc.tensor.matmul(
                                ps[:], wmat[kc][:, wsl], xin[kc][:, tsl],
                                start=(kc == 0), stop=(kc == KC - 1))
                        nc.scalar.activation(
                            dst[t][:, tsl], ps[:], FT.Identity,
                            bias=biases[:, bcol * KC + t : bcol * KC + t + 1],
                            scale=1.0)

            # garbage-pad column P2 of PKR/PQ: zero it
            for t in range(KC):
                nc.vector.memset(PKR[t][:, P2:P2 + 1], 0.0)
                nc.vector.memset(PQ[t][:, P2:P2 + 1], 0.0)

            # V (+ ones cols): out[tok_chunk, dout] = vT^T @ Wv
            for c in range(8 if "noproj" not in _ABL else 0):
                csl = slice(128 * c, 128 * (c + 1))
                ps = psp.tile([128, D], f32, name="ps_v", tag="ps_v", bufs=2)
                for osl in (slice(0, 512), slice(512, D)):
                    for kc in range(KC):
                        nc.tensor.matmul(
                            ps[:, osl], vT[kc][:, csl], wv[kc][:, osl],
                            start=(kc == 0), stop=(kc == KC - 1))
                # strided evac: VA[c][:, 65h + d] = ps[:, 64h + d]
                va_v = VA[c][:].rearrange("p (h c) -> p h c", c=65)
                ps_v = ps[:].rearrange("p (h c) -> p h c", c=64)
                nc.vector.tensor_copy(va_v[:, :, 0:64], ps_v)
                nc.vector.memset(va_v[:, :, 64:65], 1.0)

        # ---------------- Stage A: attention ----------------
        # software-pipelined: band matmuls + DRAM roundtrip for job i+1 are
        # emitted BEFORE the scores/exp/PV of job i, so the (in-order) PE
        # works on the next band while the previous skew round-trip lands.
        # ctx stays transposed ([65, tok] per head incl. the exp-sum row);
        # normalization + final transpose happen on the host.
        with ExitStack() as st:
            dram = st.enter_context(
                tc.tile_pool(name="dramb", bufs=_bufs("dramb", 2), space="DRAM"))
            bsb = st.enter_context(
                tc.tile_pool(name="bsb", bufs=_bufs("bsb", 2)))
            brd = st.enter_context(
                tc.tile_pool(name="brd", bufs=_bufs("brd", 2)))
            expp = st.enter_context(
                tc.tile_pool(name="expp", bufs=_bufs("expp", 2)))
            smal = st.enter_context(tc.tile_pool(name="smal", bufs=2))
            ps_band = st.enter_context(
                tc.tile_pool(name="ps_band", bufs=_bufs("ps_band", 2),
                             space="PSUM"))
            ps_sc = st.enter_context(
                tc.tile_pool(name="ps_sc", bufs=_bufs("ps_sc", 2),
                             space="PSUM"))
            ps_ctx = st.enter_context(
                tc.tile_pool(name="ps_ctx", bufs=_bufs("ps_ctx", 2),
                             space="PSUM"))

            RS = (slice(0, 64), slice(64, 128))

            def emit_band(b, hp):
                tok0 = 512 * b
                qh, kh, pkr, pq = QT[hp], KT[hp], PKR[hp], PQ[hp]
                cb_d = [dram.tile([4, 128, BW], bdt, name=f"cb_d{s}",
                                  tag=f"cb{s}") for s in range(2)]
                pb_d = [dram.tile([4, 128, BW], bdt, name=f"pb_d{s}",
                                  tag=f"pb{s}") for s in range(2)]
                bbm = [[bsb.tile([128, 4 * BW], bdt, name=f"bb{kind}{s}",
                                 tag=f"bb{kind}{s}") for s in range(2)]
                       for kind in range(2)]

                def _band_mm(kind, idx, s, ps, half):
                    if kind == 0:
                        w0 = 384 - 128 * idx
                        lhsT = qh[RS[s], tok0 + 128 * idx :
                                  tok0 + 128 * (idx + 1)]
                        rhs = pkr
                    else:
                        w0 = 385 - 128 * idx
                        lhsT = kh[RS[s], tok0 + 128 * idx :
                                  tok0 + 128 * (idx + 1)]
                        rhs = pq
                    nc.tensor.matmul(
                        ps[:, half], lhsT,
                        rhs[RS[s], w0 + half.start : w0 + half.stop],
                        start=True, stop=True)

                halves = (slice(0, 512), slice(512, BW))
                for kind in range(2):
                    for idx in range(4):
                        pss = [ps_band.tile([128, BW], f32, name=f"ps_b{s}",
                                            tag="ps_band")
                               for s in range(2)]
                        for half in halves:
                            for s in range(2):
                                _band_mm(kind, idx, s, pss[s], half)
                        for s in range(2):
                            # alternate the evacuation between DVE and ACT
                            # so neither engine gates the band pipeline
                            bb = bbm[kind][s]
                            o = idx * BW
                            if (idx + s) % 2:
                                nc.scalar.activation(bb[:, o : o + BW],
                                                     pss[s][:], FT.Copy)
                            else:
                                nc.vector.tensor_copy(bb[:, o : o + BW],
                                                      pss[s][:])
                    for s in range(2):
                        bd = cb_d[s] if kind == 0 else pb_d[s]
                        dst = bass.AP(bd.tensor, bd.offset,
                                      [[BW, 128], [128 * BW, 4], [1, BW]])
                        nc.sync.dma_start(dst, bbm[kind][s][:])

                # skewed (diagonal) re-reads: one batched DMA per (kind, s)
                cbrm = []
                pbrm = []
                for s in range(2):
                    for (lst, bd, nm) in ((cbrm, cb_d[s], "cbr"),
                                          (pbrm, pb_d[s], "pbr")):
                        t_ = brd.tile([128, 2048], bdt, name=f"{nm}{s}",
                                      tag=f"{nm}{s}")
                        src = bass.AP(bd.tensor, bd.offset + 127,
                                      [[BP, 128], [128 * BW, 4], [1, 512]])
                        nc.sync.dma_start(t_[:], src)
                        lst.append(t_)
                return cbrm, pbrm

            def emit_scores(b, hp, cbrm, pbrm):
                tok0 = 512 * b
                qh, kh = QT[hp], KT[hp]
                exps = [[None] * 4, [None] * 4]
                for J in range(4):
                    pss = [ps_sc.tile([128, 512], f32, name=f"ps_s{s}",
                                      tag="ps_s") for s in range(2)]
                    # c2c for both heads issued adjacently: K=64 matmuls at
                    # base partitions 0/64 overlap on the PE (row strips)
                    for s in range(2):
                        nc.tensor.matmul(
                            pss[s][:],
                            kh[RS[s], tok0 + 128 * J : tok0 + 128 * (J + 1)],
                            qh[RS[s], tok0:tok0 + 512],
                            start=True, stop=False)
                    for s in range(2):
                        nc.tensor.matmul(pss[s][:], idn[:],
                                         pbrm[s][:, 512 * J : 512 * (J + 1)],
                                         start=False, stop=False)
                        for I in range(4):
                            nc.tensor.matmul(
                                pss[s][:, 128 * I : 128 * (I + 1)],
                                cbrm[s][:, 512 * I + 128 * J :
                                          512 * I + 128 * (J + 1)],
                                idn[:], start=False, stop=(I == 3))
                        e = expp.tile([128, 512], bf16, name=f"exps{s}{J}",
                                      tag=f"exps{s}{J}")
                        nc.scalar.activation(e[:], pss[s][:], FT.Exp)
                        exps[s][J] = e

                # PV (ones-augmented): ctx^T [65, tok] per head, both heads
                # packed side-by-side and stored via one DMA; host divides
                # by the sums row and transposes
                ctxo = smal.tile([65, 1024], f32, name="ctxo", tag="ctxo")
                for s in range(2):
                    h = 2 * hp + s
                    pc = ps_ctx.tile([65, 512], f32, name="pc", tag="pc")
                    for J in range(4):
                        nc.tensor.matmul(
                            pc[:], VA[4 * b + J][:, 65 * h : 65 * h + 65],
                            exps[s][J][:], start=(J == 0), stop=(J == 3))
                    nc.vector.tensor_copy(ctxo[:, 512 * s : 512 * (s + 1)],
                                          pc[:])
                dst = bass.AP(out_d, (b * H + 2 * hp) * 65 * 512,
                              [[512, 65], [65 * 512, 2], [1, 512]])
                nc.sync.dma_start(dst, ctxo[:])

            jobs = [(b, hp) for b in range(NB) for hp in range(H // 2)]
            pending = None
            for (b, hp) in jobs:
                cur = emit_band(b, hp)
                if pending is not None:
                    emit_scores(*pending)
                pending = (b, hp, *cur)
            emit_scores(*pending)


_NC_CACHE = None
LAST = {}


def _get_nc():
    global _NC_CACHE
    if _NC_CACHE is None:
        _NC_CACHE = build_kernel()
    return _NC_CACHE


def kernel(q, k, v, rel_embeddings, Wq, bq, Wk, bk, Wv, bv, relative_pos,
           **_unused):
    q = np.asarray(q, np.float32)
    k = np.asarray(k, np.float32)
    v = np.asarray(v, np.float32)
    rel = np.asarray(rel_embeddings, np.float32)
    Wq = np.asarray(Wq, np.float32)
    Wk = np.asarray(Wk, np.float32)
    Wv = np.asarray(Wv, np.float32)
    bq = np.asarray(bq, np.float32)
    bk = np.asarray(bk, np.float32)
    bv = np.asarray(bv, np.float32)

    Wq_s, bq_s = Wq / SCALE, bq / SCALE
    wq_b = Wq_s.astype(_nbf)
    wk_b = Wk.astype(_nbf)
    wv_b = Wv.astype(_nbf)
    rT = np.ascontiguousarray(rel.T).astype(_nbf)
    rTr = np.ascontiguousarray(rel[::-1].T).astype(_nbf)
    biases = np.stack([bq_s.reshape(KC, 128), bk.reshape(KC, 128)], 0)
    biases = np.ascontiguousarray(
        biases.reshape(2 * KC, 128).T).astype(np.float32)  # [128, 2*KC]

    in_maps = []
    for c in range(NCORES):
        bs = [NB * c + i for i in range(NB)]
        qT = np.ascontiguousarray(
            np.concatenate([q[b].T for b in bs], axis=1)).astype(_nbf)
        kT = np.ascontiguousarray(
            np.concatenate([k[b].T for b in bs], axis=1)).astype(_nbf)
        vT = np.ascontiguousarray(
            np.concatenate([v[b].T for b in bs], axis=1)).astype(_nbf)
        in_maps.append({
            "qT": qT, "kT": kT, "vT": vT,
            "Wq": wq_b, "Wk": wk_b, "Wv": wv_b,
            "rT": rT, "rTrev": rTr, "biases": biases,
        })

    nc = _get_nc()
    res = bass_utils.run_bass_kernel_spmd(
        nc, in_maps, core_ids=list(range(NCORES)),
        trace=bool(int(os.environ.get("KTRACE", "0"))))
    LAST["res"] = res
    out = np.empty((B, S, D), np.float32)
    for c in range(NCORES):
        o = res.results[c]["out"].reshape(NB, H, 65, S)
        ctx = o[:, :, 0:64, :] / o[:, :, 64:65, :]  # [NB, H, HD, S]
        for i in range(NB):
            out[NB * c + i] = ctx[i].transpose(2, 0, 1).reshape(S, D)
    return out


if __name__ == "__main__":
    nc = build_kernel()
    print("built ok")



# revision 18
# speedup vs baseline: 1.1869x; 1.1869x over previous
"""Trainium2 Bass kernel for DeBERTa-style disentangled attention.

Problem: B=16, S=512, D=768, H=12, HD=64, L=512 (att_span), scale=sqrt(3*64).

  Q = q@Wq+bq, K = k@Wk+bk, V = v@Wv+bv   (per-head split)
  scores = (QK^T + c2p + p2c) / scale ; softmax ; ctx = P@V
  c2p[i,j] = Q[i] . pos_k[i-j+512]   (pos_k = rel@Wk+bk, per head)
  p2c[i,j] = K[j] . pos_q[i-j+512]   (pos_q = rel@Wq+bq)
  (clip never binds: i-j+512 in [1,1023])

Sharding: data-parallel over batch, 8 cores x (B_local=2).

Device strategy (per core, everything transposed "scores^T[j,i]"):
  - Projections produce QT/KT [dout, tok] (bf16), V [tok, dout] (bf16,
    augmented with a ones-column per head for softmax denominators),
    PKR = pos_k_reversed^T and PQ = pos_q^T [dout, p] (bf16).
    1/scale folded into Wq/bq on host (covers c2c, c2p via Q; p2c via pos_q).
  - Per (b,h): band matmuls produce c2p_att_rev / p2c_att [128, 640] tiles,
    evacuated bf16 and round-tripped through DRAM; strided re-read
    (row-stride 639, offset 127) yields the diagonal-gathered
    c2p [i,j] / p2cT [j,i] tiles (the DeBERTa "skew" trick).
  - scores^T accumulated in PSUM: c2cT matmul + p2cT via identity-add +
    c2p via PE add-transpose (lhsT=c2p chunk, rhs=identity).
  - exp on ACT (no max subtraction needed: |scores| <~ 3), PV matmul with
    ones-augmented V gives ctx^T and row sums; PE transpose + DVE
    reciprocal/scale finishes ctx = P@V / sums in fp32.
"""

import os
import sys
import numpy as np

for p in ("/opt/trn_rl_repo",):
    if p not in sys.path:
        sys.path.insert(0, p)

import ml_dtypes

import concourse.bass as bass
import concourse.bacc as bacc
import concourse.tile as tile
import concourse.mybir as mybir
from concourse import bass_utils

f32 = mybir.dt.float32
bf16 = mybir.dt.bfloat16
fp8 = mybir.dt.float8e4
FT = mybir.ActivationFunctionType

B, S, D, H = 16, 512, 768, 12
HD = 64
L = 512
P2 = 2 * L  # 1024
NB = 2  # batches per core
NTOK = NB * S  # 1024
NCORES = 8
SCALE = float(np.sqrt(HD * 3.0))
KC = D // 128  # 6 contraction chunks
BW = 640  # band width (pads the 639 used diagonals)
BP = 639  # band row pitch for the skew read

_nbf = ml_dtypes.bfloat16

# ablation / tuning knobs (TimelineSim experiments)
_ABL = set(os.environ.get("KABL", "").split(",")) - {""}
_BUFS = {}
for _kv in os.environ.get("KBUFS", "").split(","):
    if _kv:
        _k, _v = _kv.split("=")
        _BUFS[_k] = int(_v)


def _bufs(name, default):
    return _BUFS.get(name, default)


def build_kernel(abl=None, bufs=None, nrep=1):
    global _ABL, _BUFS
    if abl is not None:
        _ABL = set(abl)
    if bufs is not None:
        _BUFS = dict(bufs)
    nc = bacc.Bacc(
        "TRN2",
        target_bir_lowering=False,
        debug=False,
        enable_asserts=False,
        num_devices=NCORES,
    )

    # ---- I/O ----
    qT_d = nc.dram_tensor("qT", [D, NTOK], bf16, kind="ExternalInput")
    kT_d = nc.dram_tensor("kT", [D, NTOK], bf16, kind="ExternalInput")
    vT_d = nc.dram_tensor("vT", [D, NTOK], bf16, kind="ExternalInput")
    wq_d = nc.dram_tensor("Wq", [D, D], bf16, kind="ExternalInput")  # pre-scaled
    wk_d = nc.dram_tensor("Wk", [D, D], bf16, kind="ExternalInput")
    wv_d = nc.dram_tensor("Wv", [D, D], bf16, kind="ExternalInput")
    rT_d = nc.dram_tensor("rT", [D, P2], bf16, kind="ExternalInput")
    rTr_d = nc.dram_tensor("rTrev", [D, P2], bf16, kind="ExternalInput")
    bias_d = nc.dram_tensor("biases", [128, 2 * KC], f32, kind="ExternalInput")
    # ctx^T per (batch, head): [65, 512] (64 dims + exp-sum row);
    # host divides by sums and transposes
    out_d = nc.dram_tensor("out", [NB * H * 65, 512], f32,
                           kind="ExternalOutput")

    if "bf16rt" in _ABL:
        idn_np = np.eye(128, dtype=_nbf)
    else:
        idn_np = np.eye(128, dtype=ml_dtypes.float8_e4m3)
    idn_d = nc.inline_tensor(idn_np, name="idn_bf")

    with tile.TileContext(nc) as tc:
        for _rep in range(nrep):
            _body(nc, tc, qT_d, kT_d, vT_d, wq_d, wk_d, wv_d, rT_d, rTr_d,
                  bias_d, idn_d, out_d)
    nc.compile()
    return nc


def _body(nc, tc, qT_d, kT_d, vT_d, wq_d, wk_d, wv_d, rT_d, rTr_d,
          bias_d, idn_d, out_d):
    from contextlib import ExitStack

    with ExitStack() as big:
        const = big.enter_context(tc.tile_pool(name="const", bufs=1))
        acts = big.enter_context(tc.tile_pool(name="acts", bufs=1))

        # fp8 band round-trip by default: halves the skew-gather DRAM
        # traffic; measured end-to-end rel-err ~0.9% (vs 0.37% bf16),
        # within the 2% gate
        bdt = bf16 if "bf16rt" in _ABL else fp8
        idn = const.tile([128, 128], bdt)
        nc.sync.dma_start(idn[:], idn_d.ap())
        biases = const.tile([128, 2 * KC], f32)
        nc.sync.dma_start(biases[:], bias_d.ap())

        # persistent activations
        QT = [acts.tile([128, NTOK], bf16, name=f"QT{t}") for t in range(KC)]
        KT = [acts.tile([128, NTOK], bf16, name=f"KT{t}") for t in range(KC)]
        PKR = [acts.tile([128, P2 + 1], bf16, name=f"PKR{t}") for t in range(KC)]
        PQ = [acts.tile([128, P2 + 1], bf16, name=f"PQ{t}") for t in range(KC)]
        VA = [acts.tile([128, 65 * H], bf16, name=f"VA{c}") for c in range(8)]

        # ---------------- Stage P: projections ----------------
        with ExitStack() as st:
            inp = st.enter_context(tc.tile_pool(name="inp", bufs=1))
            psp = st.enter_context(
                tc.tile_pool(name="psp", bufs=4, space="PSUM"))

            # merged input tiles: one DMA per tensor; chunk kc lives at
            # free-dim offset kc*ncols (src dims ordered (p, kc, col))
            def _load_merged(name, src_d, ncols):
                t_ = inp.tile([128, KC * ncols], bf16, name=name)
                src = bass.AP(src_d, 0,
                              [[ncols, 128], [128 * ncols, KC], [1, ncols]])
                nc.sync.dma_start(t_[:], src)
                return [t_[:, ncols * kc: ncols * (kc + 1)] for kc in range(KC)]

            # load order: weights first so the Q/K projections can start
            # while the rest of the inputs stream in; V path last (consumed
            # latest)
            wq = _load_merged("wqm", wq_d, D)
            wk = _load_merged("wkm", wk_d, D)
            qT = _load_merged("qTm", qT_d, NTOK)
            kTt = _load_merged("kTm", kT_d, NTOK)
            rTr = _load_merged("rTrm", rTr_d, P2)
            rT = _load_merged("rTm", rT_d, P2)
            wv = _load_merged("wvm", wv_d, D)
            vT = _load_merged("vTm", vT_d, NTOK)

            # QT / KT / PKR / PQ : out[dout_tile, tok] = W^T @ xT (+ bias)
            if "noproj" in _ABL:
                for t in range(KC):
                    nc.vector.memset(QT[t][:], 0.0)
                    nc.vector.memset(KT[t][:], 0.0)
                    nc.vector.memset(PKR[t][:], 0.0)
                    nc.vector.memset(PQ[t][:], 0.0)
                for c in range(8):
                    nc.vector.memset(VA[c][:], 0.0)
            for t in range(KC if "noproj" not in _ABL else 0):
                wsl = slice(128 * t, 128 * (t + 1))
                for th in range(2):  # token/pos halves of 512
                    tsl = slice(512 * th, 512 * (th + 1))
                    for (wmat, xin, bcol, dst) in (
                        (wq, qT, 0, QT), (wk, kTt, 1, KT),
                        (wk, rTr, 1, PKR), (wq, rT, 0, PQ),
                    ):
                        ps = psp.tile([128, 512], f32, name="ps_proj",
                                      tag="ps_proj", bufs=4)
                        for kc in range(KC):
                            nc.tensor.matmul(
                                ps[:], wmat[kc][:, wsl], xin[kc][:, tsl],
                                start=(kc == 0), stop=(kc == KC - 1))
                        nc.scalar.activation(
                            dst[t][:, tsl], ps[:], FT.Identity,
                            bias=biases[:, bcol * KC + t : bcol * KC + t + 1],
                            scale=1.0)

            # garbage-pad column P2 of PKR/PQ: zero it
            for t in range(KC):
                nc.vector.memset(PKR[t][:, P2:P2 + 1], 0.0)
                nc.vector.memset(PQ[t][:, P2:P2 + 1], 0.0)

            # V (+ ones cols): out[tok_chunk, dout] = vT^T @ Wv
            for c in range(8 if "noproj" not in _ABL else 0):
                csl = slice(128 * c, 128 * (c + 1))
                ps = psp.tile([128, D], f32, name="ps_v", tag="ps_v", bufs=2)
                for osl in (slice(0, 512), slice(512, D)):
                    for kc in range(KC):
                        nc.tensor.matmul(
                            ps[:, osl], vT[kc][:, csl], wv[kc][:, osl],
                            start=(kc == 0), stop=(kc == KC - 1))
                # strided evac: VA[c][:, 65h + d] = ps[:, 64h + d]
                va_v = VA[c][:].rearrange("p (h c) -> p h c", c=65)
                ps_v = ps[:].rearrange("p (h c) -> p h c", c=64)
                nc.vector.tensor_copy(va_v[:, :, 0:64], ps_v)
                nc.vector.memset(va_v[:, :, 64:65], 1.0)

        # ---------------- Stage A: attention ----------------
        # software-pipelined: band matmuls + DRAM roundtrip for job i+1 are
        # emitted BEFORE the scores/exp/PV of job i, so the (in-order) PE
        # works on the next band while the previous skew round-trip lands.
        # ctx stays transposed ([65, tok] per head incl. the exp-sum row);
        # normalization + final transpose happen on the host.
        with ExitStack() as st:
            dram = st.enter_context(
                tc.tile_pool(name="dramb", bufs=_bufs("dramb", 3), space="DRAM"))
            bsb = st.enter_context(
                tc.tile_pool(name="bsb", bufs=_bufs("bsb", 3)))
            brd = st.enter_context(
                tc.tile_pool(name="brd", bufs=_bufs("brd", 3)))
            expp = st.enter_context(
                tc.tile_pool(name="expp", bufs=_bufs("expp", 2)))
            smal = st.enter_context(tc.tile_pool(name="smal", bufs=2))
            ps_band = st.enter_context(
                tc.tile_pool(name="ps_band", bufs=_bufs("ps_band", 2),
                             space="PSUM"))
            ps_sc = st.enter_context(
                tc.tile_pool(name="ps_sc", bufs=_bufs("ps_sc", 2),
                             space="PSUM"))
            ps_ctx = st.enter_context(
                tc.tile_pool(name="ps_ctx", bufs=_bufs("ps_ctx", 2),
                             space="PSUM"))

            RS = (slice(0, 64), slice(64, 128))

            def emit_band(b, hp):
                tok0 = 512 * b
                qh, kh, pkr, pq = QT[hp], KT[hp], PKR[hp], PQ[hp]
                cb_d = [dram.tile([4, 128, BW], bdt, name=f"cb_d{s}",
                                  tag=f"cb{s}") for s in range(2)]
                pb_d = [dram.tile([4, 128, BW], bdt, name=f"pb_d{s}",
                                  tag=f"pb{s}") for s in range(2)]
                bbm = [[bsb.tile([128, 4 * BW], bdt, name=f"bb{kind}{s}",
                                 tag=f"bb{kind}{s}") for s in range(2)]
                       for kind in range(2)]

                def _band_mm(kind, idx, s, ps, half):
                    if kind == 0:
                        w0 = 384 - 128 * idx
                        lhsT = qh[RS[s], tok0 + 128 * idx :
                                  tok0 + 128 * (idx + 1)]
                        rhs = pkr
                    else:
                        w0 = 385 - 128 * idx
                        lhsT = kh[RS[s], tok0 + 128 * idx :
                                  tok0 + 128 * (idx + 1)]
                        rhs = pq
                    nc.tensor.matmul(
                        ps[:, half], lhsT,
                        rhs[RS[s], w0 + half.start : w0 + half.stop],
                        start=True, stop=True)

                halves = (slice(0, 512), slice(512, BW))
                for kind in range(2):
                    for idx in range(4):
                        pss = [ps_band.tile([128, BW], f32, name=f"ps_b{s}",
                                            tag="ps_band")
                               for s in range(2)]
                        for half in halves:
                            for s in range(2):
                                _band_mm(kind, idx, s, pss[s], half)
                        for s in range(2):
                            # alternate the evacuation between DVE and ACT
                            # so neither engine gates the band pipeline
                            bb = bbm[kind][s]
                            o = idx * BW
                            if (idx + s) % 2:
                                nc.scalar.activation(bb[:, o : o + BW],
                                                     pss[s][:], FT.Copy)
                            else:
                                nc.vector.tensor_copy(bb[:, o : o + BW],
                                                      pss[s][:])
                    for s in range(2):
                        bd = cb_d[s] if kind == 0 else pb_d[s]
                        dst = bass.AP(bd.tensor, bd.offset,
                                      [[BW, 128], [128 * BW, 4], [1, BW]])
                        nc.sync.dma_start(dst, bbm[kind][s][:])

                # skewed (diagonal) re-reads: one batched DMA per (kind, s)
                cbrm = []
                pbrm = []
                for s in range(2):
                    for (lst, bd, nm) in ((cbrm, cb_d[s], "cbr"),
                                          (pbrm, pb_d[s], "pbr")):
                        t_ = brd.tile([128, 2048], bdt, name=f"{nm}{s}",
                                      tag=f"{nm}{s}")
                        src = bass.AP(bd.tensor, bd.offset + 127,
                                      [[BP, 128], [128 * BW, 4], [1, 512]])
                        nc.sync.dma_start(t_[:], src)
                        lst.append(t_)
                return cbrm, pbrm

            def emit_scores(b, hp, cbrm, pbrm):
                tok0 = 512 * b
                qh, kh = QT[hp], KT[hp]
                exps = [[None] * 4, [None] * 4]
                for J in range(4):
                    pss = [ps_sc.tile([128, 512], f32, name=f"ps_s{s}",
                                      tag="ps_s") for s in range(2)]
                    # c2c for both heads issued adjacently: K=64 matmuls at
                    # base partitions 0/64 overlap on the PE (row strips)
                    for s in range(2):
                        nc.tensor.matmul(
                            pss[s][:],
                            kh[RS[s], tok0 + 128 * J : tok0 + 128 * (J + 1)],
                            qh[RS[s], tok0:tok0 + 512],
                            start=True, stop=False)
                    for s in range(2):
                        nc.tensor.matmul(pss[s][:], idn[:],
                                         pbrm[s][:, 512 * J : 512 * (J + 1)],
                                         start=False, stop=False)
                        for I in range(4):
                            nc.tensor.matmul(
                                pss[s][:, 128 * I : 128 * (I + 1)],
                                cbrm[s][:, 512 * I + 128 * J :
                                          512 * I + 128 * (J + 1)],
                                idn[:], start=False, stop=(I == 3))
                        e = expp.tile([128, 512], bf16, name=f"exps{s}{J}",
                                      tag=f"exps{s}{J}")
                        nc.scalar.activation(e[:], pss[s][:], FT.Exp)
                        exps[s][J] = e

                # PV (ones-augmented): ctx^T [65, tok] per head, both heads
                # packed side-by-side and stored via one DMA; host divides
                # by the sums row and transposes
                ctxo = smal.tile([65, 1024], f32, name="ctxo", tag="ctxo")
                for s in range(2):
                    h = 2 * hp + s
                    pc = ps_ctx.tile([65, 512], f32, name="pc", tag="pc")
                    for J in range(4):
                        nc.tensor.matmul(
                            pc[:], VA[4 * b + J][:, 65 * h : 65 * h + 65],
                            exps[s][J][:], start=(J == 0), stop=(J == 3))
                    nc.vector.tensor_copy(ctxo[:, 512 * s : 512 * (s + 1)],
                                          pc[:])
                dst = bass.AP(out_d, (b * H + 2 * hp) * 65 * 512,
                              [[512, 65], [65 * 512, 2], [1, 512]])
                nc.sync.dma_start(dst, ctxo[:])

            # depth-2 pipeline: scores(i) runs after band(i+1) AND band(i+2)
            # are queued on the PE, giving job i's skew round-trip two full
            # band phases (~8us) to land -> no PE stall, HAM stays warm
            depth = int(os.environ.get("KDEPTH", "2"))
            jobs = [(b, hp) for b in range(NB) for hp in range(H // 2)]
            pending = []
            for (b, hp) in jobs:
                cur = emit_band(b, hp)
                pending.append((b, hp, *cur))
                if len(pending) > depth:
                    emit_scores(*pending.pop(0))
            for p in pending:
                emit_scores(*p)


_NC_CACHE = None
LAST = {}


def _get_nc():
    global _NC_CACHE
    if _NC_CACHE is None:
        _NC_CACHE = build_kernel()
    return _NC_CACHE


def kernel(q, k, v, rel_embeddings, Wq, bq, Wk, bk, Wv, bv, relative_pos,
           **_unused):
    q = np.asarray(q, np.float32)
    k = np.asarray(k, np.float32)
    v = np.asarray(v, np.float32)
    rel = np.asarray(rel_embeddings, np.float32)
    Wq = np.asarray(Wq, np.float32)
    Wk = np.asarray(Wk, np.float32)
    Wv = np.asarray(Wv, np.float32)
    bq = np.asarray(bq, np.float32)
    bk = np.asarray(bk, np.float32)
    bv = np.asarray(bv, np.float32)

    Wq_s, bq_s = Wq / SCALE, bq / SCALE
    wq_b = Wq_s.astype(_nbf)
    wk_b = Wk.astype(_nbf)
    wv_b = Wv.astype(_nbf)
    rT = np.ascontiguousarray(rel.T).astype(_nbf)
    rTr = np.ascontiguousarray(rel[::-1].T).astype(_nbf)
    biases = np.stack([bq_s.reshape(KC, 128), bk.reshape(KC, 128)], 0)
    biases = np.ascontiguousarray(
        biases.reshape(2 * KC, 128).T).astype(np.float32)  # [128, 2*KC]

    in_maps = []
    for c in range(NCORES):
        bs = [NB * c + i for i in range(NB)]
        qT = np.ascontiguousarray(
            np.concatenate([q[b].T for b in bs], axis=1)).astype(_nbf)
        kT = np.ascontiguousarray(
            np.concatenate([k[b].T for b in bs], axis=1)).astype(_nbf)
        vT = np.ascontiguousarray(
            np.concatenate([v[b].T for b in bs], axis=1)).astype(_nbf)
        in_maps.append({
            "qT": qT, "kT": kT, "vT": vT,
            "Wq": wq_b, "Wk": wk_b, "Wv": wv_b,
            "rT": rT, "rTrev": rTr, "biases": biases,
        })

    nc = _get_nc()
    res = bass_utils.run_bass_kernel_spmd(
        nc, in_maps, core_ids=list(range(NCORES)),
        trace=bool(int(os.environ.get("KTRACE", "0"))))
    LAST["res"] = res
    out = np.empty((B, S, D), np.float32)
    for c in range(NCORES):
        o = res.results[c]["out"].reshape(NB, H, 65, S)
        ctx = o[:, :, 0:64, :] / o[:, :, 64:65, :]  # [NB, H, HD, S]
        for i in range(NB):
            out[NB * c + i] = ctx[i].transpose(2, 0, 1).reshape(S, D)
    return out


if __name__ == "__main__":
    nc = build_kernel()
    print("built ok")

